# revision 30
# baseline (speedup 1.0000x reference)
"""Trainium2 Bass kernel for InputProjection + time/sensor masking + LayerNorm.

Reference computation (B=64, T=4096, C=51, D=64):
    mask[b,t,c] = time_mask[b,t] | sensor_mask[b,c]
    out = LN( einsum('btc,cd->btd', x*(1-mask), W) + einsum('btc,cd->btd', mask, Wm) )

Algebraic restructure (exact):
    With W_b[c,d]   = (1 - sm[b,c]) * W[c,d]
         smWm_b[d]  = sum_c sm[b,c]*Wm[c,d]
         allWm[d]   = sum_c Wm[c,d]
    pre[b,t,d] = sum_c x[b,t,c]*(1-tm[b,t]) * W_b[c,d]
               + 1 * smWm_b[d]
               + tm[b,t] * (allWm - smWm_b)[d]
    LayerNorm is invariant to per-token constants, so centering every row of
    the augmented weight matrix over d makes pre exactly mean-free per token:
    LN(pre) = pre * rsqrt(var+eps) with no mean subtraction / bias.

    Tokens with tm=1 all produce the SAME output row LN(allWm) -> host fills
    those directly; the device only processes the ~70% unmasked tokens,
    compacted per batch into T'=3072 columns (fallback to T'=4096 full path
    if any batch exceeds that; the reference mask density makes that
    essentially impossible).

Device kernel (per core, data-parallel over batch; bf16 in/out, fp32 PSUM):
    - xaug[pair, 128, T']: rows 0..50 = gathered x.T, row 51 = 1.0, row 52 =
      tm (zeros when compacted); second batch of the pair at rows 64..116
      (full 128-partition DMAs: partition-sliced DMAs don't round-robin
      across the 16 DMA queues). Token order is chunk-major (column j*128+m
      holds compacted token m*nj+j) so lhsT slices are contiguous (enables
      the PE fast-weight-load path).
    - per 128-token chunk: one 53-contraction bf16 matmul (stationary = x
      chunk [53,128], moving = per-batch weights [53,64]) -> PSUM fp32.
    - per PSUM tile (HB chunks, fully independent chain): ACT Square-copy
      PSUM->SBUF bf16, DVE multi-group tensor_reduce (sum of squares per
      token), ACT Sqrt (scale=1/D, bias=eps), DVE fast reciprocal -> s[t],
      DVE tensor_tensor broadcast multiply (PSUM * s -> bf16 out), DMA out.
      No per-chunk instructions anywhere; GPSIMD untouched (its tensor ops
      run ~1.2us per chunk and contend for the DVE SBUF port).
    gamma/beta are applied on host only if nontrivial (reference uses 1/0).
"""

import os
import sys
from contextlib import ExitStack

import numpy as np
import ml_dtypes

for _p in ("/opt/trn_rl_repo", "/root/.axon_site/_ro/trn_rl_repo"):
    if os.path.isdir(_p) and _p not in sys.path:
        sys.path.insert(0, _p)

import concourse.bass as bass
import concourse.bacc as bacc
import concourse.mybir as mybir
from concourse import tile
from concourse.bass_utils import run_bass_kernel_spmd

F32 = mybir.dt.float32
BF16 = mybir.dt.bfloat16
AF = mybir.ActivationFunctionType
ALU = mybir.AluOpType
BF16NP = ml_dtypes.bfloat16

B, T, C, D = 64, 4096, 51, 64
LN_EPS = 1e-5
N_CORES = 8
BPC = B // N_CORES          # batches per core
NPAIR = BPC // 2            # batch pairs per core
CAUG = C + 2                # x rows + ones row + tm row
MTILE = 128                 # tokens per matmul chunk (psum partitions)
TCOMP = 3072                # compacted token budget per batch
FOLD = os.environ.get("KV5_FOLD", "0") == "1"
OUT_DMA_ACT = os.environ.get("KV5_OUT_ACT", "0") == "1"


def build_nc(npair: int, t_len: int, debug: bool = False):
    """Build the per-core Bass program. Identical on all cores (SPMD)."""
    nj = t_len // MTILE                 # chunks per batch
    hb = 24 if nj == 24 else 16         # chunks per PSUM tile
    assert nj % hb == 0
    ntile = nj // hb

    nc = bacc.Bacc("TRN2", target_bir_lowering=False, debug=debug)
    xaug_d = nc.dram_tensor("xaug", [npair, 128, t_len], BF16,
                            kind="ExternalInput")
    waug_d = nc.dram_tensor("waug", [npair, 128, D], BF16,
                            kind="ExternalInput")
    out_d = nc.dram_tensor("out", [2 * npair, t_len, D], BF16,
                           kind="ExternalOutput")

    with tile.TileContext(nc) as tc, ExitStack() as ctx:
        wpool = ctx.enter_context(tc.tile_pool(name="wpool", bufs=1))
        xpool = ctx.enter_context(tc.tile_pool(name="xpool", bufs=8))
        opool = ctx.enter_context(tc.tile_pool(name="opool", bufs=6))
        qpool = ctx.enter_context(tc.tile_pool(name="qpool", bufs=4))
        tpool = ctx.enter_context(tc.tile_pool(name="tpool", bufs=8))
        psum = ctx.enter_context(tc.tile_pool(name="psum", bufs=2 if hb == 24 else 4, space="PSUM"))

        wa = wpool.tile([128, npair, D], BF16)
        # weights ride the Scalar engine's DGE so their descriptor
        # generation overlaps the first x transfer on SP
        nc.scalar.dma_start(wa[:], waug_d.rearrange("n k d -> k n d"))
        epst = wpool.tile([128, 1], F32)
        nc.vector.memset(epst[:], LN_EPS)

        # all input DMAs issued upfront on SP; pair 0 split in halves so
        # the first matmuls start sooner, later pairs whole (fewer
        # descriptors)
        span = (t_len // ntile)
        xas = {}
        for p in range(npair):
            if p == 0:
                for h in range(ntile):
                    xah = xpool.tile([128, span], BF16, tag=f"xa{h}")
                    nc.sync.dma_start(xah[:],
                                      xaug_d[p, :, h * span:(h + 1) * span])
                    xas[(p, h)] = xah
            else:
                xa = xpool.tile([128, t_len], BF16, tag="xaw")
                nc.sync.dma_start(xa[:], xaug_d[p])
                for h in range(ntile):
                    xas[(p, h)] = None
                xas[p] = xa

        for p in range(npair):
            for i in range(2):
                b = 2 * p + i
                rb = 64 * i
                outb = out_d[b].rearrange("(k j) d -> k j d", k=128)
                for h in range(ntile):
                    hs = slice(h * hb, (h + 1) * hb)
                    if xas[(p, h)] is not None:
                        xa = xas[(p, h)]
                        xoff = 0
                    else:
                        xa = xas[p]
                        xoff = h * span
                    ps = psum.tile([128, hb, D], F32, tag="ps")
                    for q in range(hb):
                        nc.tensor.matmul(
                            ps[:, q, :],
                            xa[rb:rb + CAUG,
                               xoff + q * MTILE:xoff + (q + 1) * MTILE],
                            wa[rb:rb + CAUG, p, :],
                            start=True,
                            stop=True,
                        )
                    sqt = qpool.tile([128, hb, D], BF16, tag="sq")
                    rs = tpool.tile([128, hb], F32, tag="rs")
                    sv = tpool.tile([128, hb], F32, tag="sv")
                    s = tpool.tile([128, hb], F32, tag="s")
                    ob = opool.tile([128, hb, D], BF16, tag="ob")
                    nc.scalar.activation(sqt[:], ps[:], AF.Square)
                    nc.vector.tensor_reduce(rs[:], sqt[:],
                                            mybir.AxisListType.X, ALU.add)
                    nc.scalar.activation(sv[:], rs[:], AF.Sqrt,
                                         bias=epst[:], scale=1.0 / D)
                    nc.vector.reciprocal_approx_fast(out=s[:], in_=sv[:])
                    nc.vector.tensor_tensor(
                        ob[:], ps[:],
                        s[:].to_broadcast([128, hb, D]),
                        ALU.mult)
                    nc.sync.dma_start(outb[:, hs, :], ob[:])
    nc.compile()
    return nc


def _center_rows_bf16(w):
    """Center rows over d in fp64, round to bf16, and iterate so the bf16
    values themselves have (near-)zero row means."""
    w = w.astype(np.float64)
    for _ in range(3):
        w = w - w.mean(axis=-1, keepdims=True)
        wb = w.astype(BF16NP)
        w = wb.astype(np.float64)
    return wb


def _host_prep(x, W, Wm, time_mask, sensor_mask, n_cores, idx, t_dev):
    """Shard along batch; gather/transpose/augment/center per-core inputs."""
    b, t_len, c = x.shape
    d = W.shape[1]
    npair = b // n_cores // 2
    nj = t_dev // MTILE

    tm = np.ascontiguousarray(time_mask).astype(np.float32)
    sm = np.ascontiguousarray(sensor_mask).astype(np.float32)
    x = np.asarray(x, dtype=np.float32)
    W = np.asarray(W, dtype=np.float64)
    Wm = np.asarray(Wm, dtype=np.float64)

    if idx is not None:
        xg = np.take_along_axis(x, idx[:, :, None], axis=1)   # [b, t_dev, c]
        tmg = np.zeros((b, t_dev), np.float32)
    else:
        xg = x * (1.0 - tm)[:, :, None]
        tmg = tm

    xaug = np.zeros((b // 2, 128, t_dev), np.float32)
    xgp = xg.reshape(b // 2, 2, t_dev, c)
    tmp_ = tmg.reshape(b // 2, 2, t_dev)
    for i in range(2):
        rb = 64 * i
        xaug[:, rb:rb + c] = xgp[:, i].transpose(0, 2, 1)
        xaug[:, rb + c] = 1.0
        xaug[:, rb + c + 1] = tmp_[:, i]
    # chunk-major token permutation: column j*128+m <- token m*nj+j
    xaug = (xaug.reshape(b // 2, 128, MTILE, nj)
                .transpose(0, 1, 3, 2)
                .reshape(b // 2, 128, t_dev))
    xaug = xaug.astype(BF16NP)

    allWm = Wm.sum(axis=0)
    smWm = sm.astype(np.float64) @ Wm
    waug_c = np.empty((b, CAUG, d), np.float64)
    waug_c[:, :c] = W[None] * (1.0 - sm.astype(np.float64))[:, :, None]
    waug_c[:, c] = smWm
    waug_c[:, c + 1] = allWm[None] - smWm
    waug_c = _center_rows_bf16(waug_c)
    waug = np.zeros((b // 2, 128, d), BF16NP)
    waug[:, 0:CAUG] = waug_c[0::2]
    waug[:, 64:64 + CAUG] = waug_c[1::2]

    in_maps = []
    for m in range(n_cores):
        slp = slice(m * npair, (m + 1) * npair)
        in_maps.append({
            "xaug": np.ascontiguousarray(xaug[slp]),
            "waug": np.ascontiguousarray(waug[slp]),
        })
    return in_maps


_NC_CACHE = {}


def kernel(x, W, Wm, gamma, beta, time_mask, sensor_mask):
    x = np.asarray(x)
    b, t_len, c = x.shape
    n_cores = N_CORES
    npair = b // n_cores // 2

    tm = np.ascontiguousarray(time_mask).astype(bool)
    counts = (~tm).sum(axis=1)
    compact = (t_len % 1024 == 0 and TCOMP < t_len
               and counts.max() <= TCOMP)
    if compact:
        t_dev = TCOMP
        order = np.argsort(tm, axis=1, kind="stable")
        idx = np.ascontiguousarray(order[:, :TCOMP])
    else:
        t_dev = t_len
        idx = None

    key = (npair, t_dev)
    if key not in _NC_CACHE:
        _NC_CACHE[key] = build_nc(npair, t_dev)
    nc = _NC_CACHE[key]

    in_maps = _host_prep(x, W, Wm, tm, sensor_mask, n_cores, idx, t_dev)

    trace = bool(int(os.environ.get("KERNEL_TRACE", "0")))
    res = run_bass_kernel_spmd(nc, in_maps, list(range(n_cores)), trace=trace)
    kernel.last_results = res

    dev = np.concatenate(
        [np.asarray(res.results[i]["out"]) for i in range(n_cores)], axis=0
    ).astype(np.float32)

    if compact:
        # masked tokens: constant row LN(allWm); kept tokens: scatter back
        Wm64 = np.asarray(Wm, dtype=np.float64)
        allWm = Wm64.sum(axis=0)
        v = allWm - allWm.mean()
        crow = (v / np.sqrt((v ** 2).mean() + LN_EPS)).astype(np.float32)
        out = np.empty((b, t_len, D), np.float32)
        out[tm] = crow
        for bi in range(b):
            n = counts[bi]
            out[bi, idx[bi, :n]] = dev[bi, :n]
    else:
        out = dev

    gamma = np.asarray(gamma, dtype=np.float32)
    beta = np.asarray(beta, dtype=np.float32)
    if not (np.all(gamma == 1.0) and np.all(beta == 0.0)):
        out = out * gamma + beta
    return out


# revision 32
# speedup vs baseline: 1.0446x; 1.0446x over previous
"""Trainium2 Bass kernel for InputProjection + time/sensor masking + LayerNorm.

Reference computation (B=64, T=4096, C=51, D=64):
    mask[b,t,c] = time_mask[b,t] | sensor_mask[b,c]
    out = LN( einsum('btc,cd->btd', x*(1-mask), W) + einsum('btc,cd->btd', mask, Wm) )

Algebraic restructure (exact):
    With W_b[c,d]   = (1 - sm[b,c]) * W[c,d]
         smWm_b[d]  = sum_c sm[b,c]*Wm[c,d]
         allWm[d]   = sum_c Wm[c,d]
    pre[b,t,d] = sum_c x[b,t,c]*(1-tm[b,t]) * W_b[c,d]
               + 1 * smWm_b[d]
               + tm[b,t] * (allWm - smWm_b)[d]
    LayerNorm is invariant to per-token constants, so centering every row of
    the augmented weight matrix over d makes pre exactly mean-free per token:
    LN(pre) = pre * rsqrt(var+eps) with no mean subtraction / bias.

    Tokens with tm=1 all produce the SAME output row LN(allWm) -> host fills
    those directly; the device only processes the ~70% unmasked tokens,
    compacted per batch into T'=3072 columns (fallback to T'=4096 full path
    if any batch exceeds that; the reference mask density makes that
    essentially impossible).

Device kernel (per core, data-parallel over batch; bf16 in/out, fp32 PSUM):
    - xaug[pair, 128, T']: rows 0..50 = gathered x.T, row 51 = 1.0, row 52 =
      tm (zeros when compacted); second batch of the pair at rows 64..116
      (full 128-partition DMAs: partition-sliced DMAs don't round-robin
      across the 16 DMA queues). Token order is chunk-major (column j*128+m
      holds compacted token m*nj+j) so lhsT slices are contiguous (enables
      the PE fast-weight-load path).
    - per 128-token chunk: one 53-contraction bf16 matmul (stationary = x
      chunk [53,128], moving = per-batch weights [53,64]) -> PSUM fp32.
    - per PSUM tile (HB chunks, fully independent chain): ACT Square-copy
      PSUM->SBUF bf16, DVE multi-group tensor_reduce (sum of squares per
      token), ACT Sqrt (scale=1/D, bias=eps), DVE fast reciprocal -> s[t],
      DVE tensor_tensor broadcast multiply (PSUM * s -> bf16 out), DMA out.
      No per-chunk instructions anywhere; GPSIMD untouched (its tensor ops
      run ~1.2us per chunk and contend for the DVE SBUF port).
    gamma/beta are applied on host only if nontrivial (reference uses 1/0).
"""

import os
import sys
from contextlib import ExitStack

import numpy as np
import ml_dtypes

for _p in ("/opt/trn_rl_repo", "/root/.axon_site/_ro/trn_rl_repo"):
    if os.path.isdir(_p) and _p not in sys.path:
        sys.path.insert(0, _p)

import concourse.bass as bass
import concourse.bacc as bacc
import concourse.mybir as mybir
from concourse import tile
from concourse.bass_utils import run_bass_kernel_spmd

F32 = mybir.dt.float32
BF16 = mybir.dt.bfloat16
AF = mybir.ActivationFunctionType
ALU = mybir.AluOpType
BF16NP = ml_dtypes.bfloat16

B, T, C, D = 64, 4096, 51, 64
LN_EPS = 1e-5
N_CORES = 8
BPC = B // N_CORES          # batches per core
NPAIR = BPC // 2            # batch pairs per core
CAUG = C + 2                # x rows + ones row + tm row
MTILE = 128                 # tokens per matmul chunk (psum partitions)
TCOMP = 3072                # compacted token budget per batch
FOLD = os.environ.get("KV5_FOLD", "0") == "1"
OUT_DMA_ACT = os.environ.get("KV5_OUT_ACT", "0") == "1"


def build_nc(npair: int, t_len: int, debug: bool = False):
    """Build the per-core Bass program. Identical on all cores (SPMD)."""
    nj = t_len // MTILE                 # chunks per batch
    hb = 16 if nj % 16 == 0 else 12     # chunks per PSUM tile
    assert nj % hb == 0
    ntile = nj // hb

    nc = bacc.Bacc("TRN2", target_bir_lowering=False, debug=debug)
    xaug_d = nc.dram_tensor("xaug", [npair, 128, t_len], BF16,
                            kind="ExternalInput")
    waug_d = nc.dram_tensor("waug", [npair, 128, D], BF16,
                            kind="ExternalInput")
    out_d = nc.dram_tensor("out", [2 * npair, t_len, D], BF16,
                           kind="ExternalOutput")

    with tile.TileContext(nc) as tc, ExitStack() as ctx:
        wpool = ctx.enter_context(tc.tile_pool(name="wpool", bufs=1))
        xpool = ctx.enter_context(tc.tile_pool(name="xpool", bufs=8))
        opool = ctx.enter_context(tc.tile_pool(name="opool", bufs=6))
        qpool = ctx.enter_context(tc.tile_pool(name="qpool", bufs=4))
        tpool = ctx.enter_context(tc.tile_pool(name="tpool", bufs=8))
        psum = ctx.enter_context(tc.tile_pool(name="psum", bufs=4, space="PSUM"))

        wa = wpool.tile([128, npair, D], BF16)
        # weights ride the Scalar engine's DGE so their descriptor
        # generation overlaps the first x transfer on SP
        nc.scalar.dma_start(wa[:], waug_d.rearrange("n k d -> k n d"))
        epst = wpool.tile([128, 1], F32)
        nc.vector.memset(epst[:], LN_EPS)

        # all input DMAs issued upfront on SP; pair 0 split in halves so
        # the first matmuls start sooner, later pairs whole (fewer
        # descriptors)
        span = (t_len // ntile)
        xas = {}
        for p in range(npair):
            if p == 0:
                for h in range(ntile):
                    xah = xpool.tile([128, span], BF16, tag=f"xa{h}")
                    nc.sync.dma_start(xah[:],
                                      xaug_d[p, :, h * span:(h + 1) * span])
                    xas[(p, h)] = xah
            else:
                xa = xpool.tile([128, t_len], BF16, tag="xaw")
                nc.sync.dma_start(xa[:], xaug_d[p])
                for h in range(ntile):
                    xas[(p, h)] = None
                xas[p] = xa

        for p in range(npair):
            for i in range(2):
                b = 2 * p + i
                rb = 64 * i
                outb = out_d[b].rearrange("(k j) d -> k j d", k=128)
                for h in range(ntile):
                    hs = slice(h * hb, (h + 1) * hb)
                    if xas[(p, h)] is not None:
                        xa = xas[(p, h)]
                        xoff = 0
                    else:
                        xa = xas[p]
                        xoff = h * span
                    ps = psum.tile([128, hb, D], F32, tag="ps")
                    for q in range(hb):
                        nc.tensor.matmul(
                            ps[:, q, :],
                            xa[rb:rb + CAUG,
                               xoff + q * MTILE:xoff + (q + 1) * MTILE],
                            wa[rb:rb + CAUG, p, :],
                            start=True,
                            stop=True,
                        )
                    sqt = qpool.tile([128, hb, D], BF16, tag="sq")
                    rs = tpool.tile([128, hb], F32, tag="rs")
                    sv = tpool.tile([128, hb], F32, tag="sv")
                    s = tpool.tile([128, hb], F32, tag="s")
                    ob = opool.tile([128, hb, D], BF16, tag="ob")
                    if p == 0 and i == 0 and h == 0:
                        # split the very first square/reduce so DVE starts
                        # after 6 matmuls instead of 12 (shaves the head)
                        hh = hb // 2
                        nc.scalar.activation(sqt[:, :hh, :], ps[:, :hh, :],
                                             AF.Square)
                        nc.vector.tensor_reduce(rs[:, :hh], sqt[:, :hh, :],
                                                mybir.AxisListType.X, ALU.add)
                        nc.scalar.activation(sqt[:, hh:, :], ps[:, hh:, :],
                                             AF.Square)
                        nc.vector.tensor_reduce(rs[:, hh:], sqt[:, hh:, :],
                                                mybir.AxisListType.X, ALU.add)
                    else:
                        nc.scalar.activation(sqt[:], ps[:], AF.Square)
                        nc.vector.tensor_reduce(rs[:], sqt[:],
                                                mybir.AxisListType.X, ALU.add)
                    nc.scalar.activation(sv[:], rs[:], AF.Sqrt,
                                         bias=epst[:], scale=1.0 / D)
                    nc.vector.reciprocal_approx_fast(out=s[:], in_=sv[:])
                    nc.vector.tensor_tensor(
                        ob[:], ps[:],
                        s[:].to_broadcast([128, hb, D]),
                        ALU.mult)
                    if p == npair - 1 and i == 1 and h == ntile - 1:
                        # split the last output DMA so descriptor generation
                        # overlaps the first half's transfer (shaves the tail)
                        hh = hb // 2
                        nc.sync.dma_start(outb[:, h * hb:h * hb + hh, :],
                                          ob[:, :hh, :])
                        nc.sync.dma_start(outb[:, h * hb + hh:(h + 1) * hb, :],
                                          ob[:, hh:, :])
                    else:
                        nc.sync.dma_start(outb[:, hs, :], ob[:])
    nc.compile()
    return nc


def _center_rows_bf16(w):
    """Center rows over d in fp64, round to bf16, and iterate so the bf16
    values themselves have (near-)zero row means."""
    w = w.astype(np.float64)
    for _ in range(3):
        w = w - w.mean(axis=-1, keepdims=True)
        wb = w.astype(BF16NP)
        w = wb.astype(np.float64)
    return wb


def _host_prep(x, W, Wm, time_mask, sensor_mask, n_cores, idx, t_dev):
    """Shard along batch; gather/transpose/augment/center per-core inputs."""
    b, t_len, c = x.shape
    d = W.shape[1]
    npair = b // n_cores // 2
    nj = t_dev // MTILE

    tm = np.ascontiguousarray(time_mask).astype(np.float32)
    sm = np.ascontiguousarray(sensor_mask).astype(np.float32)
    x = np.asarray(x, dtype=np.float32)
    W = np.asarray(W, dtype=np.float64)
    Wm = np.asarray(Wm, dtype=np.float64)

    if idx is not None:
        xg = np.take_along_axis(x, idx[:, :, None], axis=1)   # [b, t_dev, c]
        tmg = np.zeros((b, t_dev), np.float32)
    else:
        xg = x * (1.0 - tm)[:, :, None]
        tmg = tm

    xaug = np.zeros((b // 2, 128, t_dev), np.float32)
    xgp = xg.reshape(b // 2, 2, t_dev, c)
    tmp_ = tmg.reshape(b // 2, 2, t_dev)
    for i in range(2):
        rb = 64 * i
        xaug[:, rb:rb + c] = xgp[:, i].transpose(0, 2, 1)
        xaug[:, rb + c] = 1.0
        xaug[:, rb + c + 1] = tmp_[:, i]
    # chunk-major token permutation: column j*128+m <- token m*nj+j
    xaug = (xaug.reshape(b // 2, 128, MTILE, nj)
                .transpose(0, 1, 3, 2)
                .reshape(b // 2, 128, t_dev))
    xaug = xaug.astype(BF16NP)

    allWm = Wm.sum(axis=0)
    smWm = sm.astype(np.float64) @ Wm
    waug_c = np.empty((b, CAUG, d), np.float64)
    waug_c[:, :c] = W[None] * (1.0 - sm.astype(np.float64))[:, :, None]
    waug_c[:, c] = smWm
    waug_c[:, c + 1] = allWm[None] - smWm
    waug_c = _center_rows_bf16(waug_c)
    waug = np.zeros((b // 2, 128, d), BF16NP)
    waug[:, 0:CAUG] = waug_c[0::2]
    waug[:, 64:64 + CAUG] = waug_c[1::2]

    in_maps = []
    for m in range(n_cores):
        slp = slice(m * npair, (m + 1) * npair)
        in_maps.append({
            "xaug": np.ascontiguousarray(xaug[slp]),
            "waug": np.ascontiguousarray(waug[slp]),
        })
    return in_maps


_NC_CACHE = {}


def kernel(x, W, Wm, gamma, beta, time_mask, sensor_mask):
    x = np.asarray(x)
    b, t_len, c = x.shape
    n_cores = N_CORES
    npair = b // n_cores // 2

    tm = np.ascontiguousarray(time_mask).astype(bool)
    counts = (~tm).sum(axis=1)
    compact = (t_len % 1024 == 0 and TCOMP < t_len
               and counts.max() <= TCOMP)
    if compact:
        t_dev = TCOMP
        order = np.argsort(tm, axis=1, kind="stable")
        idx = np.ascontiguousarray(order[:, :TCOMP])
    else:
        t_dev = t_len
        idx = None

    key = (npair, t_dev)
    if key not in _NC_CACHE:
        _NC_CACHE[key] = build_nc(npair, t_dev)
    nc = _NC_CACHE[key]

    in_maps = _host_prep(x, W, Wm, tm, sensor_mask, n_cores, idx, t_dev)

    trace = bool(int(os.environ.get("KERNEL_TRACE", "0")))
    res = run_bass_kernel_spmd(nc, in_maps, list(range(n_cores)), trace=trace)
    kernel.last_results = res

    dev = np.concatenate(
        [np.asarray(res.results[i]["out"]) for i in range(n_cores)], axis=0
    ).astype(np.float32)

    if compact:
        # masked tokens: constant row LN(allWm); kept tokens: scatter back
        Wm64 = np.asarray(Wm, dtype=np.float64)
        allWm = Wm64.sum(axis=0)
        v = allWm - allWm.mean()
        crow = (v / np.sqrt((v ** 2).mean() + LN_EPS)).astype(np.float32)
        out = np.empty((b, t_len, D), np.float32)
        out[tm] = crow
        for bi in range(b):
            n = counts[bi]
            out[bi, idx[bi, :n]] = dev[bi, :n]
    else:
        out = dev

    gamma = np.asarray(gamma, dtype=np.float32)
    beta = np.asarray(beta, dtype=np.float32)
    if not (np.all(gamma == 1.0) and np.all(beta == 0.0)):
        out = out * gamma + beta
    return out


# revision 34
# speedup vs baseline: 1.0922x; 1.0456x over previous
"""Trainium2 Bass kernel for InputProjection + time/sensor masking + LayerNorm.

Reference computation (B=64, T=4096, C=51, D=64):
    mask[b,t,c] = time_mask[b,t] | sensor_mask[b,c]
    out = LN( einsum('btc,cd->btd', x*(1-mask), W) + einsum('btc,cd->btd', mask, Wm) )

Algebraic restructure (exact):
    With W_b[c,d]   = (1 - sm[b,c]) * W[c,d]
         smWm_b[d]  = sum_c sm[b,c]*Wm[c,d]
         allWm[d]   = sum_c Wm[c,d]
    pre[b,t,d] = sum_c x[b,t,c]*(1-tm[b,t]) * W_b[c,d]
               + 1 * smWm_b[d]
               + tm[b,t] * (allWm - smWm_b)[d]
    LayerNorm is invariant to per-token constants, so centering every row of
    the augmented weight matrix over d makes pre exactly mean-free per token:
    LN(pre) = pre * rsqrt(var+eps) with no mean subtraction / bias.

    Tokens with tm=1 all produce the SAME output row LN(allWm) -> host fills
    those directly; the device only processes the ~70% unmasked tokens,
    compacted per batch into T'=3072 columns (fallback to T'=4096 full path
    if any batch exceeds that; the reference mask density makes that
    essentially impossible).

Device kernel (per core, data-parallel over batch; bf16 in/out, fp32 PSUM):
    - xaug[pair, 128, T']: rows 0..50 = gathered x.T, row 51 = 1.0, row 52 =
      tm (zeros when compacted); second batch of the pair at rows 64..116
      (full 128-partition DMAs: partition-sliced DMAs don't round-robin
      across the 16 DMA queues). Token order is chunk-major (column j*128+m
      holds compacted token m*nj+j) so lhsT slices are contiguous (enables
      the PE fast-weight-load path).
    - per 128-token chunk: one 53-contraction bf16 matmul (stationary = x
      chunk [53,128], moving = per-batch weights [53,64]) -> PSUM fp32.
    - per PSUM tile (HB chunks, fully independent chain): ACT Square-copy
      PSUM->SBUF bf16, DVE multi-group tensor_reduce (sum of squares per
      token), ACT Sqrt (scale=1/D, bias=eps), DVE fast reciprocal -> s[t],
      DVE tensor_tensor broadcast multiply (PSUM * s -> bf16 out), DMA out.
      No per-chunk instructions anywhere; GPSIMD untouched (its tensor ops
      run ~1.2us per chunk and contend for the DVE SBUF port).
    gamma/beta are applied on host only if nontrivial (reference uses 1/0).
"""

import os
import sys
from contextlib import ExitStack

import numpy as np
import ml_dtypes

for _p in ("/opt/trn_rl_repo", "/root/.axon_site/_ro/trn_rl_repo"):
    if os.path.isdir(_p) and _p not in sys.path:
        sys.path.insert(0, _p)

import concourse.bass as bass
import concourse.bacc as bacc
import concourse.mybir as mybir
from concourse import tile
from concourse.bass_utils import run_bass_kernel_spmd

F32 = mybir.dt.float32
BF16 = mybir.dt.bfloat16
AF = mybir.ActivationFunctionType
ALU = mybir.AluOpType
BF16NP = ml_dtypes.bfloat16

B, T, C, D = 64, 4096, 51, 64
LN_EPS = 1e-5
N_CORES = 8
BPC = B // N_CORES          # batches per core
NPAIR = BPC // 2            # batch pairs per core
CAUG = C + 2                # x rows + ones row + tm row
MTILE = 128                 # tokens per matmul chunk (psum partitions)
TCOMP = 3072                # compacted token budget per batch


def build_nc(npair: int, t_len: int, debug: bool = False):
    """Build the per-core Bass program. Identical on all cores (SPMD)."""
    nj = t_len // MTILE                 # chunks per batch
    hb = 16 if nj % 16 == 0 else 12     # chunks per PSUM tile
    assert nj % hb == 0
    ntile = nj // hb

    nc = bacc.Bacc("TRN2", target_bir_lowering=False, debug=debug)
    xaug_d = nc.dram_tensor("xaug", [npair, 128, t_len], BF16,
                            kind="ExternalInput")
    waug_d = nc.dram_tensor("waug", [npair, 128, D], BF16,
                            kind="ExternalInput")
    out_d = nc.dram_tensor("out", [2 * npair, t_len, D], BF16,
                           kind="ExternalOutput")

    with tile.TileContext(nc) as tc, ExitStack() as ctx:
        wpool = ctx.enter_context(tc.tile_pool(name="wpool", bufs=1))
        xpool = ctx.enter_context(tc.tile_pool(name="xpool", bufs=8))
        opool = ctx.enter_context(tc.tile_pool(name="opool", bufs=6))
        qpool = ctx.enter_context(tc.tile_pool(name="qpool", bufs=4))
        tpool = ctx.enter_context(tc.tile_pool(name="tpool", bufs=8))
        psum = ctx.enter_context(tc.tile_pool(name="psum", bufs=4, space="PSUM"))

        wa = wpool.tile([128, npair, D], BF16)
        # weights ride the Scalar engine's DGE so their descriptor
        # generation overlaps the first x transfer on SP
        nc.scalar.dma_start(wa[:], waug_d.rearrange("n k d -> k n d"))
        epst = wpool.tile([128, 1], F32)
        nc.vector.memset(epst[:], LN_EPS)

        # all input DMAs issued upfront on SP; pair 0 split in halves so
        # the first matmuls start sooner, later pairs whole (fewer
        # descriptors)
        span = (t_len // ntile)
        xas = {}
        for p in range(npair):
            if p == 0:
                for h in range(ntile):
                    xah = xpool.tile([128, span], BF16, tag=f"xa{h}")
                    nc.sync.dma_start(xah[:],
                                      xaug_d[p, :, h * span:(h + 1) * span])
                    xas[(p, h)] = xah
            else:
                xa = xpool.tile([128, t_len], BF16, tag="xaw")
                nc.sync.dma_start(xa[:], xaug_d[p])
                for h in range(ntile):
                    xas[(p, h)] = None
                xas[p] = xa

        for p in range(npair):
            for i in range(2):
                b = 2 * p + i
                rb = 64 * i
                outb = out_d[b].rearrange("(k j) d -> k j d", k=128)
                for h in range(ntile):
                    hs = slice(h * hb, (h + 1) * hb)
                    if xas[(p, h)] is not None:
                        xa = xas[(p, h)]
                        xoff = 0
                    else:
                        xa = xas[p]
                        xoff = h * span
                    ps = psum.tile([128, hb, D], F32, tag="ps")
                    for q in range(hb):
                        nc.tensor.matmul(
                            ps[:, q, :],
                            xa[rb:rb + CAUG,
                               xoff + q * MTILE:xoff + (q + 1) * MTILE],
                            wa[rb:rb + CAUG, p, :],
                            start=True,
                            stop=True,
                        )
                    sqt = qpool.tile([128, hb, D], BF16, tag="sq")
                    rs = tpool.tile([128, hb], F32, tag="rs")
                    sv = tpool.tile([128, hb], F32, tag="sv")
                    s = tpool.tile([128, hb], F32, tag="s")
                    ob = opool.tile([128, hb, D], BF16, tag="ob")
                    nc.scalar.activation(sqt[:], ps[:], AF.Square)
                    nc.vector.tensor_reduce(rs[:], sqt[:],
                                            mybir.AxisListType.X, ALU.add)
                    nc.scalar.activation(sv[:], rs[:], AF.Sqrt,
                                         bias=epst[:], scale=1.0 / D)
                    nc.vector.reciprocal_approx_fast(out=s[:], in_=sv[:])
                    nc.vector.tensor_tensor(
                        ob[:], ps[:],
                        s[:].to_broadcast([128, hb, D]),
                        ALU.mult)
                    nc.sync.dma_start(outb[:, hs, :], ob[:])
    nc.compile()
    return nc


def _center_rows_bf16(w):
    """Center rows over d in fp64, round to bf16, and iterate so the bf16
    values themselves have (near-)zero row means."""
    w = w.astype(np.float64)
    for _ in range(3):
        w = w - w.mean(axis=-1, keepdims=True)
        wb = w.astype(BF16NP)
        w = wb.astype(np.float64)
    return wb


def _host_prep(x, W, Wm, time_mask, sensor_mask, n_cores, idx, t_dev):
    """Shard along batch; gather/transpose/augment/center per-core inputs."""
    b, t_len, c = x.shape
    d = W.shape[1]
    npair = b // n_cores // 2
    nj = t_dev // MTILE

    tm = np.ascontiguousarray(time_mask).astype(np.float32)
    sm = np.ascontiguousarray(sensor_mask).astype(np.float32)
    x = np.asarray(x, dtype=np.float32)
    W = np.asarray(W, dtype=np.float64)
    Wm = np.asarray(Wm, dtype=np.float64)

    if idx is not None:
        xg = np.take_along_axis(x, idx[:, :, None], axis=1)   # [b, t_dev, c]
        tmg = np.zeros((b, t_dev), np.float32)
    else:
        xg = x * (1.0 - tm)[:, :, None]
        tmg = tm

    xaug = np.zeros((b // 2, 128, t_dev), np.float32)
    xgp = xg.reshape(b // 2, 2, t_dev, c)
    tmp_ = tmg.reshape(b // 2, 2, t_dev)
    for i in range(2):
        rb = 64 * i
        xaug[:, rb:rb + c] = xgp[:, i].transpose(0, 2, 1)
        xaug[:, rb + c] = 1.0
        xaug[:, rb + c + 1] = tmp_[:, i]
    # chunk-major token permutation: column j*128+m <- token m*nj+j
    xaug = (xaug.reshape(b // 2, 128, MTILE, nj)
                .transpose(0, 1, 3, 2)
                .reshape(b // 2, 128, t_dev))
    xaug = xaug.astype(BF16NP)

    allWm = Wm.sum(axis=0)
    smWm = sm.astype(np.float64) @ Wm
    waug_c = np.empty((b, CAUG, d), np.float64)
    waug_c[:, :c] = W[None] * (1.0 - sm.astype(np.float64))[:, :, None]
    waug_c[:, c] = smWm
    waug_c[:, c + 1] = allWm[None] - smWm
    waug_c = _center_rows_bf16(waug_c)
    waug = np.zeros((b // 2, 128, d), BF16NP)
    waug[:, 0:CAUG] = waug_c[0::2]
    waug[:, 64:64 + CAUG] = waug_c[1::2]

    in_maps = []
    for m in range(n_cores):
        slp = slice(m * npair, (m + 1) * npair)
        in_maps.append({
            "xaug": np.ascontiguousarray(xaug[slp]),
            "waug": np.ascontiguousarray(waug[slp]),
        })
    return in_maps


_NC_CACHE = {}


def kernel(x, W, Wm, gamma, beta, time_mask, sensor_mask):
    x = np.asarray(x)
    b, t_len, c = x.shape
    n_cores = N_CORES
    npair = b // n_cores // 2

    tm = np.ascontiguousarray(time_mask).astype(bool)
    counts = (~tm).sum(axis=1)
    compact = (t_len % 1024 == 0 and TCOMP < t_len
               and counts.max() <= TCOMP)
    if compact:
        t_dev = TCOMP
        order = np.argsort(tm, axis=1, kind="stable")
        idx = np.ascontiguousarray(order[:, :TCOMP])
    else:
        t_dev = t_len
        idx = None

    key = (npair, t_dev)
    if key not in _NC_CACHE:
        _NC_CACHE[key] = build_nc(npair, t_dev)
    nc = _NC_CACHE[key]

    in_maps = _host_prep(x, W, Wm, tm, sensor_mask, n_cores, idx, t_dev)

    trace = bool(int(os.environ.get("KERNEL_TRACE", "0")))
    res = run_bass_kernel_spmd(nc, in_maps, list(range(n_cores)), trace=trace)
    kernel.last_results = res

    dev = np.concatenate(
        [np.asarray(res.results[i]["out"]) for i in range(n_cores)], axis=0
    ).astype(np.float32)

    if compact:
        # masked tokens: constant row LN(allWm); kept tokens: scatter back
        Wm64 = np.asarray(Wm, dtype=np.float64)
        allWm = Wm64.sum(axis=0)
        v = allWm - allWm.mean()
        crow = (v / np.sqrt((v ** 2).mean() + LN_EPS)).astype(np.float32)
        out = np.empty((b, t_len, D), np.float32)
        out[tm] = crow
        for bi in range(b):
            n = counts[bi]
            out[bi, idx[bi, :n]] = dev[bi, :n]
    else:
        out = dev

    gamma = np.asarray(gamma, dtype=np.float32)
    beta = np.asarray(beta, dtype=np.float32)
    if not (np.all(gamma == 1.0) and np.all(beta == 0.0)):
        out = out * gamma + beta
    return out


# revision 36
# speedup vs baseline: 1.1134x; 1.0194x over previous
"""Trainium2 Bass kernel for InputProjection + time/sensor masking + LayerNorm.

Reference computation (B=64, T=4096, C=51, D=64):
    mask[b,t,c] = time_mask[b,t] | sensor_mask[b,c]
    out = LN( einsum('btc,cd->btd', x*(1-mask), W) + einsum('btc,cd->btd', mask, Wm) )

Algebraic restructure (exact):
    With W_b[c,d]   = (1 - sm[b,c]) * W[c,d]
         smWm_b[d]  = sum_c sm[b,c]*Wm[c,d]
         allWm[d]   = sum_c Wm[c,d]
    pre[b,t,d] = sum_c x[b,t,c]*(1-tm[b,t]) * W_b[c,d]
               + 1 * smWm_b[d]
               + tm[b,t] * (allWm - smWm_b)[d]
    LayerNorm is invariant to per-token constants, so centering every row of
    the augmented weight matrix over d makes pre exactly mean-free per token:
    LN(pre) = pre * rsqrt(var+eps) with no mean subtraction / bias.

    Tokens with tm=1 all produce the SAME output row LN(allWm) -> host fills
    those directly; the device only processes the ~70% unmasked tokens,
    compacted per batch into T'=3072 columns (fallback to T'=4096 full path
    if any batch exceeds that; the reference mask density makes that
    essentially impossible).

Device kernel (per core, data-parallel over batch; bf16 in/out, fp32 PSUM):
    - xaug[pair, 128, T']: rows 0..50 = gathered x.T, row 51 = 1.0, row 52 =
      tm (zeros when compacted); second batch of the pair at rows 64..116
      (full 128-partition DMAs: partition-sliced DMAs don't round-robin
      across the 16 DMA queues). Token order is chunk-major (column j*128+m
      holds compacted token m*nj+j) so lhsT slices are contiguous (enables
      the PE fast-weight-load path).
    - per 128-token chunk: one 53-contraction bf16 matmul (stationary = x
      chunk [53,128], moving = per-batch weights [53,64]) -> PSUM fp32.
    - per PSUM tile (HB chunks, fully independent chain): ACT Square-copy
      PSUM->SBUF bf16, DVE multi-group tensor_reduce (sum of squares per
      token), ACT Sqrt (scale=1/D, bias=eps), DVE fast reciprocal -> s[t],
      DVE tensor_tensor broadcast multiply (PSUM * s -> bf16 out), DMA out.
      No per-chunk instructions anywhere; GPSIMD untouched (its tensor ops
      run ~1.2us per chunk and contend for the DVE SBUF port).
    gamma/beta are applied on host only if nontrivial (reference uses 1/0).
"""

import os
import sys
from contextlib import ExitStack

import numpy as np
import ml_dtypes

for _p in ("/opt/trn_rl_repo", "/root/.axon_site/_ro/trn_rl_repo"):
    if os.path.isdir(_p) and _p not in sys.path:
        sys.path.insert(0, _p)

import concourse.bass as bass
import concourse.bacc as bacc
import concourse.mybir as mybir
from concourse import tile
from concourse.bass_utils import run_bass_kernel_spmd

F32 = mybir.dt.float32
BF16 = mybir.dt.bfloat16
AF = mybir.ActivationFunctionType
ALU = mybir.AluOpType
BF16NP = ml_dtypes.bfloat16

B, T, C, D = 64, 4096, 51, 64
LN_EPS = 1e-5
N_CORES = 8
BPC = B // N_CORES          # batches per core
NPAIR = BPC // 2            # batch pairs per core
CAUG = C + 2                # x rows + ones row + tm row
MTILE = 128                 # tokens per matmul chunk (psum partitions)
TCOMP = 3072                # compacted token budget per batch


def build_nc(npair: int, t_len: int, debug: bool = False):
    """Build the per-core Bass program. Identical on all cores (SPMD)."""
    nj = t_len // MTILE                 # chunks per batch
    hb = 16 if nj % 16 == 0 else 12     # chunks per PSUM tile
    assert nj % hb == 0
    ntile = nj // hb

    nc = bacc.Bacc("TRN2", target_bir_lowering=False, debug=debug)
    xaug_d = nc.dram_tensor("xaug", [npair, 128, t_len], BF16,
                            kind="ExternalInput")
    waug_d = nc.dram_tensor("waug", [npair, 128, D], BF16,
                            kind="ExternalInput")
    out_d = nc.dram_tensor("out", [2 * npair, t_len, D], BF16,
                           kind="ExternalOutput")

    with tile.TileContext(nc) as tc, ExitStack() as ctx:
        wpool = ctx.enter_context(tc.tile_pool(name="wpool", bufs=1))
        xpool = ctx.enter_context(tc.tile_pool(name="xpool", bufs=8))
        opool = ctx.enter_context(tc.tile_pool(name="opool", bufs=6))
        qpool = ctx.enter_context(tc.tile_pool(name="qpool", bufs=4))
        tpool = ctx.enter_context(tc.tile_pool(name="tpool", bufs=8))
        psum = ctx.enter_context(tc.tile_pool(name="psum", bufs=4, space="PSUM"))

        wa = wpool.tile([128, npair, D], BF16)
        # weights ride the Scalar engine's DGE so their descriptor
        # generation overlaps the first x transfer on SP
        nc.scalar.dma_start(wa[:], waug_d.rearrange("n k d -> k n d"))
        epst = wpool.tile([128, 1], F32)
        nc.vector.memset(epst[:], LN_EPS)

        # all input DMAs issued upfront on SP; pair 0 split in halves so
        # the first matmuls start sooner, later pairs whole (fewer
        # descriptors)
        span = (t_len // ntile)
        xas = {}
        for p in range(npair):
            if p == 0:
                for h in range(ntile):
                    xah = xpool.tile([128, span], BF16, tag=f"xa{h}")
                    nc.sync.dma_start(xah[:],
                                      xaug_d[p, :, h * span:(h + 1) * span])
                    xas[(p, h)] = xah
            else:
                xa = xpool.tile([128, t_len], BF16, tag="xaw")
                nc.sync.dma_start(xa[:], xaug_d[p])
                for h in range(ntile):
                    xas[(p, h)] = None
                xas[p] = xa

        for p in range(npair):
            for i in range(2):
                b = 2 * p + i
                rb = 64 * i
                outb = out_d[b].rearrange("(k j) d -> k j d", k=128)
                obb = opool.tile([128, nj, D], BF16, tag="obb")
                for h in range(ntile):
                    hs = slice(h * hb, (h + 1) * hb)
                    if xas[(p, h)] is not None:
                        xa = xas[(p, h)]
                        xoff = 0
                    else:
                        xa = xas[p]
                        xoff = h * span
                    ps = psum.tile([128, hb, D], F32, tag="ps")
                    for q in range(hb):
                        nc.tensor.matmul(
                            ps[:, q, :],
                            xa[rb:rb + CAUG,
                               xoff + q * MTILE:xoff + (q + 1) * MTILE],
                            wa[rb:rb + CAUG, p, :],
                            start=True,
                            stop=True,
                        )
                    sqt = qpool.tile([128, hb, D], BF16, tag="sq")
                    rs = tpool.tile([128, hb], F32, tag="rs")
                    sv = tpool.tile([128, hb], F32, tag="sv")
                    s = tpool.tile([128, hb], F32, tag="s")
                    nc.scalar.activation(sqt[:], ps[:], AF.Square)
                    nc.vector.tensor_reduce(rs[:], sqt[:],
                                            mybir.AxisListType.X, ALU.add)
                    nc.scalar.activation(sv[:], rs[:], AF.Sqrt,
                                         bias=epst[:], scale=1.0 / D)
                    nc.vector.reciprocal_approx_fast(out=s[:], in_=sv[:])
                    nc.vector.tensor_tensor(
                        obb[:, hs, :], ps[:],
                        s[:].to_broadcast([128, hb, D]),
                        ALU.mult)
                # one DMA per batch (3KB runs, half the descriptor count of
                # per-tile DMAs); the final batch rides the ACT DGE, whose
                # sequencer is idle by then, so the tail descriptor gen
                # doesn't queue behind SP's backlog
                eng = nc.scalar if (p == npair - 1 and i == 1) else nc.sync
                eng.dma_start(outb[:], obb[:])
    nc.compile()
    return nc


def _center_rows_bf16(w):
    """Center rows over d in fp64, round to bf16, and iterate so the bf16
    values themselves have (near-)zero row means."""
    w = w.astype(np.float64)
    for _ in range(3):
        w = w - w.mean(axis=-1, keepdims=True)
        wb = w.astype(BF16NP)
        w = wb.astype(np.float64)
    return wb


def _host_prep(x, W, Wm, time_mask, sensor_mask, n_cores, idx, t_dev):
    """Shard along batch; gather/transpose/augment/center per-core inputs."""
    b, t_len, c = x.shape
    d = W.shape[1]
    npair = b // n_cores // 2
    nj = t_dev // MTILE

    tm = np.ascontiguousarray(time_mask).astype(np.float32)
    sm = np.ascontiguousarray(sensor_mask).astype(np.float32)
    x = np.asarray(x, dtype=np.float32)
    W = np.asarray(W, dtype=np.float64)
    Wm = np.asarray(Wm, dtype=np.float64)

    if idx is not None:
        xg = np.take_along_axis(x, idx[:, :, None], axis=1)   # [b, t_dev, c]
        tmg = np.zeros((b, t_dev), np.float32)
    else:
        xg = x * (1.0 - tm)[:, :, None]
        tmg = tm

    xaug = np.zeros((b // 2, 128, t_dev), np.float32)
    xgp = xg.reshape(b // 2, 2, t_dev, c)
    tmp_ = tmg.reshape(b // 2, 2, t_dev)
    for i in range(2):
        rb = 64 * i
        xaug[:, rb:rb + c] = xgp[:, i].transpose(0, 2, 1)
        xaug[:, rb + c] = 1.0
        xaug[:, rb + c + 1] = tmp_[:, i]
    # chunk-major token permutation: column j*128+m <- token m*nj+j
    xaug = (xaug.reshape(b // 2, 128, MTILE, nj)
                .transpose(0, 1, 3, 2)
                .reshape(b // 2, 128, t_dev))
    xaug = xaug.astype(BF16NP)

    allWm = Wm.sum(axis=0)
    smWm = sm.astype(np.float64) @ Wm
    waug_c = np.empty((b, CAUG, d), np.float64)
    waug_c[:, :c] = W[None] * (1.0 - sm.astype(np.float64))[:, :, None]
    waug_c[:, c] = smWm
    waug_c[:, c + 1] = allWm[None] - smWm
    waug_c = _center_rows_bf16(waug_c)
    waug = np.zeros((b // 2, 128, d), BF16NP)
    waug[:, 0:CAUG] = waug_c[0::2]
    waug[:, 64:64 + CAUG] = waug_c[1::2]

    in_maps = []
    for m in range(n_cores):
        slp = slice(m * npair, (m + 1) * npair)
        in_maps.append({
            "xaug": np.ascontiguousarray(xaug[slp]),
            "waug": np.ascontiguousarray(waug[slp]),
        })
    return in_maps


_NC_CACHE = {}


def kernel(x, W, Wm, gamma, beta, time_mask, sensor_mask):
    x = np.asarray(x)
    b, t_len, c = x.shape
    n_cores = N_CORES
    npair = b // n_cores // 2

    tm = np.ascontiguousarray(time_mask).astype(bool)
    counts = (~tm).sum(axis=1)
    compact = (t_len % 1024 == 0 and TCOMP < t_len
               and counts.max() <= TCOMP)
    if compact:
        t_dev = TCOMP
        order = np.argsort(tm, axis=1, kind="stable")
        idx = np.ascontiguousarray(order[:, :TCOMP])
    else:
        t_dev = t_len
        idx = None

    key = (npair, t_dev)
    if key not in _NC_CACHE:
        _NC_CACHE[key] = build_nc(npair, t_dev)
    nc = _NC_CACHE[key]

    in_maps = _host_prep(x, W, Wm, tm, sensor_mask, n_cores, idx, t_dev)

    trace = bool(int(os.environ.get("KERNEL_TRACE", "0")))
    res = run_bass_kernel_spmd(nc, in_maps, list(range(n_cores)), trace=trace)
    kernel.last_results = res

    dev = np.concatenate(
        [np.asarray(res.results[i]["out"]) for i in range(n_cores)], axis=0
    ).astype(np.float32)

    if compact:
        # masked tokens: constant row LN(allWm); kept tokens: scatter back
        Wm64 = np.asarray(Wm, dtype=np.float64)
        allWm = Wm64.sum(axis=0)
        v = allWm - allWm.mean()
        crow = (v / np.sqrt((v ** 2).mean() + LN_EPS)).astype(np.float32)
        out = np.empty((b, t_len, D), np.float32)
        out[tm] = crow
        for bi in range(b):
            n = counts[bi]
            out[bi, idx[bi, :n]] = dev[bi, :n]
    else:
        out = dev

    gamma = np.asarray(gamma, dtype=np.float32)
    beta = np.asarray(beta, dtype=np.float32)
    if not (np.all(gamma == 1.0) and np.all(beta == 0.0)):
        out = out * gamma + beta
    return out


# revision 39
# speedup vs baseline: 1.1225x; 1.0082x over previous
"""Trainium2 Bass kernel for InputProjection + time/sensor masking + LayerNorm.

Reference computation (B=64, T=4096, C=51, D=64):
    mask[b,t,c] = time_mask[b,t] | sensor_mask[b,c]
    out = LN( einsum('btc,cd->btd', x*(1-mask), W) + einsum('btc,cd->btd', mask, Wm) )

Algebraic restructure (exact):
    With W_b[c,d]   = (1 - sm[b,c]) * W[c,d]
         smWm_b[d]  = sum_c sm[b,c]*Wm[c,d]
         allWm[d]   = sum_c Wm[c,d]
    pre[b,t,d] = sum_c x[b,t,c]*(1-tm[b,t]) * W_b[c,d]
               + 1 * smWm_b[d]
               + tm[b,t] * (allWm - smWm_b)[d]
    LayerNorm is invariant to per-token constants, so centering every row of
    the augmented weight matrix over d makes pre exactly mean-free per token:
    LN(pre) = pre * rsqrt(var+eps) with no mean subtraction / bias.

    Tokens with tm=1 all produce the SAME output row LN(allWm) -> host fills
    those directly; the device only processes the ~70% unmasked tokens,
    compacted per batch into T'=3072 columns (fallback to T'=4096 full path
    if any batch exceeds that; the reference mask density makes that
    essentially impossible).

Device kernel (per core, data-parallel over batch; bf16 in/out, fp32 PSUM):
    - xaug[pair, 128, T']: rows 0..50 = gathered x.T, row 51 = 1.0, row 52 =
      tm (zeros when compacted); second batch of the pair at rows 64..116
      (full 128-partition DMAs: partition-sliced DMAs don't round-robin
      across the 16 DMA queues). Token order is chunk-major (column j*128+m
      holds compacted token m*nj+j) so lhsT slices are contiguous (enables
      the PE fast-weight-load path).
    - per 128-token chunk: one 53-contraction bf16 matmul (stationary = x
      chunk [53,128], moving = per-batch weights [53,64]) -> PSUM fp32.
    - per PSUM tile (HB chunks, fully independent chain): ACT Square-copy
      PSUM->SBUF bf16, DVE multi-group tensor_reduce (sum of squares per
      token), ACT Sqrt (scale=1/D, bias=eps), DVE fast reciprocal -> s[t],
      DVE tensor_tensor broadcast multiply (PSUM * s -> bf16 out), DMA out.
      No per-chunk instructions anywhere; GPSIMD untouched (its tensor ops
      run ~1.2us per chunk and contend for the DVE SBUF port).
    gamma/beta are applied on host only if nontrivial (reference uses 1/0).
"""

import os
import sys
from contextlib import ExitStack

import numpy as np
import ml_dtypes

for _p in ("/opt/trn_rl_repo", "/root/.axon_site/_ro/trn_rl_repo"):
    if os.path.isdir(_p) and _p not in sys.path:
        sys.path.insert(0, _p)

import concourse.bass as bass
import concourse.bacc as bacc
import concourse.mybir as mybir
from concourse import tile
from concourse.bass_utils import run_bass_kernel_spmd

F32 = mybir.dt.float32
BF16 = mybir.dt.bfloat16
AF = mybir.ActivationFunctionType
ALU = mybir.AluOpType
BF16NP = ml_dtypes.bfloat16

B, T, C, D = 64, 4096, 51, 64
LN_EPS = 1e-5
N_CORES = 8
BPC = B // N_CORES          # batches per core
NPAIR = BPC // 2            # batch pairs per core
CAUG = C + 2                # x rows + ones row + tm row
MTILE = 128                 # tokens per matmul chunk (psum partitions)
TCOMP = 3072                # compacted token budget per batch


def build_nc(npair: int, t_len: int, debug: bool = False):
    """Build the per-core Bass program. Identical on all cores (SPMD)."""
    nj = t_len // MTILE                 # chunks per batch
    hb = 16 if nj % 16 == 0 else 12     # chunks per PSUM tile
    assert nj % hb == 0
    ntile = nj // hb

    nc = bacc.Bacc("TRN2", target_bir_lowering=False, debug=debug)
    xaug_d = nc.dram_tensor("xaug", [npair, 128, t_len], BF16,
                            kind="ExternalInput")
    waug_d = nc.dram_tensor("waug", [npair, 128, D], BF16,
                            kind="ExternalInput")
    out_d = nc.dram_tensor("out", [2 * npair, t_len, D], BF16,
                           kind="ExternalOutput")

    with tile.TileContext(nc) as tc, ExitStack() as ctx:
        wpool = ctx.enter_context(tc.tile_pool(name="wpool", bufs=1))
        xpool = ctx.enter_context(tc.tile_pool(name="xpool", bufs=8))
        opool = ctx.enter_context(tc.tile_pool(name="opool", bufs=6))
        qpool = ctx.enter_context(tc.tile_pool(name="qpool", bufs=4))
        tpool = ctx.enter_context(tc.tile_pool(name="tpool", bufs=8))
        psum = ctx.enter_context(tc.tile_pool(name="psum", bufs=4, space="PSUM"))

        wa = wpool.tile([128, npair, D], BF16)
        # weights ride the Scalar engine's DGE so their descriptor
        # generation overlaps the first x transfer on SP
        nc.scalar.dma_start(wa[:], waug_d.rearrange("n k d -> k n d"))
        epst = wpool.tile([128, 1], F32)
        nc.vector.memset(epst[:], LN_EPS)

        # all input DMAs issued upfront on SP; pair 0 split in halves so
        # the first matmuls start sooner, later pairs whole (fewer
        # descriptors)
        span = (t_len // ntile)
        xas = {}
        for p in range(npair):
            if p == 0:
                for h in range(ntile):
                    if h == 0:
                        # quarter-span tiles: the very first matmuls wait
                        # for only 1/4 of the pair's data
                        qs = span // 2
                        sub = []
                        for g in range(2):
                            xag = xpool.tile([128, qs], BF16, tag=f"xa0{g}")
                            nc.sync.dma_start(
                                xag[:], xaug_d[p, :, g * qs:(g + 1) * qs])
                            sub.append(xag)
                        xas[(p, h)] = sub
                    else:
                        xah = xpool.tile([128, span], BF16, tag=f"xa{h}")
                        nc.sync.dma_start(
                            xah[:], xaug_d[p, :, h * span:(h + 1) * span])
                        xas[(p, h)] = xah
            else:
                xa = xpool.tile([128, t_len], BF16, tag="xaw")
                nc.sync.dma_start(xa[:], xaug_d[p])
                for h in range(ntile):
                    xas[(p, h)] = None
                xas[p] = xa

        for p in range(npair):
            for i in range(2):
                b = 2 * p + i
                rb = 64 * i
                outb = out_d[b].rearrange("(k j) d -> k j d", k=128)
                obb = opool.tile([128, nj, D], BF16, tag="obb")
                for h in range(ntile):
                    hs = slice(h * hb, (h + 1) * hb)
                    ps = psum.tile([128, hb, D], F32, tag="ps")
                    for q in range(hb):
                        src = xas[(p, h)]
                        if src is None:
                            xa = xas[p]
                            c0 = h * span + q * MTILE
                        elif isinstance(src, list):
                            qh = hb // 2
                            xa = src[q // qh]
                            c0 = (q % qh) * MTILE
                        else:
                            xa = src
                            c0 = q * MTILE
                        nc.tensor.matmul(
                            ps[:, q, :],
                            xa[rb:rb + CAUG, c0:c0 + MTILE],
                            wa[rb:rb + CAUG, p, :],
                            start=True,
                            stop=True,
                        )
                    sqt = qpool.tile([128, hb, D], BF16, tag="sq")
                    rs = tpool.tile([128, hb], F32, tag="rs")
                    sv = tpool.tile([128, hb], F32, tag="sv")
                    s = tpool.tile([128, hb], F32, tag="s")
                    nc.scalar.activation(sqt[:], ps[:], AF.Square)
                    nc.vector.tensor_reduce(rs[:], sqt[:],
                                            mybir.AxisListType.X, ALU.add)
                    nc.scalar.activation(sv[:], rs[:], AF.Sqrt,
                                         bias=epst[:], scale=1.0 / D)
                    nc.vector.reciprocal_approx_fast(out=s[:], in_=sv[:])
                    nc.vector.tensor_tensor(
                        obb[:, hs, :], ps[:],
                        s[:].to_broadcast([128, hb, D]),
                        ALU.mult)
                # one DMA per batch (3KB runs, half the descriptor count of
                # per-tile DMAs). The final batch instead issues per-tile
                # DMAs on the ACT DGE (idle by then): the first tile's
                # descriptor gen overlaps the last tile's compute, shrinking
                # the serial tail.
                if p == npair - 1 and i == 1:
                    for h in range(ntile):
                        hs = slice(h * hb, (h + 1) * hb)
                        nc.scalar.dma_start(outb[:, hs, :], obb[:, hs, :])
                else:
                    nc.sync.dma_start(outb[:], obb[:])
    nc.compile()
    return nc


def _center_rows_bf16(w):
    """Center rows over d in fp64, round to bf16, and iterate so the bf16
    values themselves have (near-)zero row means."""
    w = w.astype(np.float64)
    for _ in range(3):
        w = w - w.mean(axis=-1, keepdims=True)
        wb = w.astype(BF16NP)
        w = wb.astype(np.float64)
    return wb


def _host_prep(x, W, Wm, time_mask, sensor_mask, n_cores, idx, t_dev):
    """Shard along batch; gather/transpose/augment/center per-core inputs."""
    b, t_len, c = x.shape
    d = W.shape[1]
    npair = b // n_cores // 2
    nj = t_dev // MTILE

    tm = np.ascontiguousarray(time_mask).astype(np.float32)
    sm = np.ascontiguousarray(sensor_mask).astype(np.float32)
    x = np.asarray(x, dtype=np.float32)
    W = np.asarray(W, dtype=np.float64)
    Wm = np.asarray(Wm, dtype=np.float64)

    if idx is not None:
        xg = np.take_along_axis(x, idx[:, :, None], axis=1)   # [b, t_dev, c]
        tmg = np.zeros((b, t_dev), np.float32)
    else:
        xg = x * (1.0 - tm)[:, :, None]
        tmg = tm

    xaug = np.zeros((b // 2, 128, t_dev), np.float32)
    xgp = xg.reshape(b // 2, 2, t_dev, c)
    tmp_ = tmg.reshape(b // 2, 2, t_dev)
    for i in range(2):
        rb = 64 * i
        xaug[:, rb:rb + c] = xgp[:, i].transpose(0, 2, 1)
        xaug[:, rb + c] = 1.0
        xaug[:, rb + c + 1] = tmp_[:, i]
    # chunk-major token permutation: column j*128+m <- token m*nj+j
    xaug = (xaug.reshape(b // 2, 128, MTILE, nj)
                .transpose(0, 1, 3, 2)
                .reshape(b // 2, 128, t_dev))
    xaug = xaug.astype(BF16NP)

    allWm = Wm.sum(axis=0)
    smWm = sm.astype(np.float64) @ Wm
    waug_c = np.empty((b, CAUG, d), np.float64)
    waug_c[:, :c] = W[None] * (1.0 - sm.astype(np.float64))[:, :, None]
    waug_c[:, c] = smWm
    waug_c[:, c + 1] = allWm[None] - smWm
    waug_c = _center_rows_bf16(waug_c)
    waug = np.zeros((b // 2, 128, d), BF16NP)
    waug[:, 0:CAUG] = waug_c[0::2]
    waug[:, 64:64 + CAUG] = waug_c[1::2]

    in_maps = []
    for m in range(n_cores):
        slp = slice(m * npair, (m + 1) * npair)
        in_maps.append({
            "xaug": np.ascontiguousarray(xaug[slp]),
            "waug": np.ascontiguousarray(waug[slp]),
        })
    return in_maps


_NC_CACHE = {}


def kernel(x, W, Wm, gamma, beta, time_mask, sensor_mask):
    x = np.asarray(x)
    b, t_len, c = x.shape
    n_cores = N_CORES
    npair = b // n_cores // 2

    tm = np.ascontiguousarray(time_mask).astype(bool)
    counts = (~tm).sum(axis=1)
    compact = (t_len % 1024 == 0 and TCOMP < t_len
               and counts.max() <= TCOMP)
    if compact:
        t_dev = TCOMP
        order = np.argsort(tm, axis=1, kind="stable")
        idx = np.ascontiguousarray(order[:, :TCOMP])
    else:
        t_dev = t_len
        idx = None

    key = (npair, t_dev)
    if key not in _NC_CACHE:
        _NC_CACHE[key] = build_nc(npair, t_dev)
    nc = _NC_CACHE[key]

    in_maps = _host_prep(x, W, Wm, tm, sensor_mask, n_cores, idx, t_dev)

    trace = bool(int(os.environ.get("KERNEL_TRACE", "0")))
    res = run_bass_kernel_spmd(nc, in_maps, list(range(n_cores)), trace=trace)
    kernel.last_results = res

    dev = np.concatenate(
        [np.asarray(res.results[i]["out"]) for i in range(n_cores)], axis=0
    ).astype(np.float32)

    if compact:
        # masked tokens: constant row LN(allWm); kept tokens: scatter back
        Wm64 = np.asarray(Wm, dtype=np.float64)
        allWm = Wm64.sum(axis=0)
        v = allWm - allWm.mean()
        crow = (v / np.sqrt((v ** 2).mean() + LN_EPS)).astype(np.float32)
        out = np.empty((b, t_len, D), np.float32)
        out[tm] = crow
        for bi in range(b):
            n = counts[bi]
            out[bi, idx[bi, :n]] = dev[bi, :n]
    else:
        out = dev

    gamma = np.asarray(gamma, dtype=np.float32)
    beta = np.asarray(beta, dtype=np.float32)
    if not (np.all(gamma == 1.0) and np.all(beta == 0.0)):
        out = out * gamma + beta
    return out
